# revision 58
# baseline (speedup 1.0000x reference)
"""Masked multi-head attention on 8 Trainium2 NeuronCores.

Sharding: core c = (b, hg) with b = c // 4, hg = c % 4. Each core computes the
full attention block for batch b restricted to heads [4*hg, 4*hg+4), including
its slice of the QKV projection and of the output projection. The host sums the
4 tensor-parallel partial outputs per batch and adds the (V-bias-folded) output
bias.

Numerics: the QKV projections run as split-precision fp8e4m3 DoubleRow
matmuls (x and the x32-scaled weights each split into hi + lo-residual;
hi*Whi + lo*Whi + hi*Wlo at 0.5 cycles/row, compensated by 1/32 during PSUM
evacuation), the QK^T score matmuls run in fp8e4m3 DoubleRow over a
[32-partition, 2-half] per-head layout, and AV / out-projection run in bf16.
K-bias is dropped (softmax row-shift invariance), V-bias is folded into the
output bias on the host, Q-bias is applied during PSUM evacuation.

Layouts (per core):
  xth/xtl    [128, 8, 2048]   fp8  x^T hi / lo residual
  wqkh/wqkl  [128, 4, 2, 512] fp8  col blocks QA|QB|KA|KB; block col 32*i+j
                                   is head (4*hg+i), dims j / 32+j (A/B
                                   half). Q pre-scaled by 1/sqrt(Hd); all
                                   QKV weights pre-scaled by 32 so the lo
                                   residual stays in fp8 normal range.
  qkt  [64, 2, 2, 2, T] fp8   [32*(h%2)+j, h//2, half, q/k, t]: per-head
                              slices sit at partition base 0/32 (PE cannot
                              address base 96)
  wvh/wvl    [128, 4, 2, 256] fp8
  vp   [128, 4, 16, 65] bf16  V tiles per (head, k-tile); col 64 = ones row
                              (softmax denominator rides in the AV matmul)
  ot   [128, 2, T]     bf16   normalized O^T; rows 64*(h%2)+d at dim1 h//2
  wout [128, 2, 1024]  bf16

Schedule: per 512-token query block, the two head pairs run causal
kt-loops with fp8 scores prefetched one tile ahead (plus a cross-pair
kt=0 prefetch), exp on the Act engine over both heads at once, bf16 AV
accumulating O^T and the softmax denominator in PSUM. QKV of block tt+1
and the (fully deferred) out-projections of earlier blocks fill the PE
dependency gaps. Softmax normalization runs off the critical path from an
SBUF copy of the AV accumulator (reciprocal on DVE, partition-broadcast
on Pool). Shapes hardcoded for B=2, T=2048, D=1024, H=16, Hd=64.
"""

import numpy as np
import ml_dtypes
from collections import deque
from contextlib import ExitStack

import concourse.bass as bass
import concourse.bacc as bacc
import concourse.mybir as mybir
import concourse.tile as tile
from concourse.bass_utils import run_bass_kernel_spmd

B, T, D = 2, 2048, 1024
H, HD = 16, 64
HL = 4               # heads per core
NCORES = 8
TQ = 512             # query tile
TK = 128             # key tile
NQT = T // TQ        # 4
NKT = T // TK        # 16
NDT = D // 128       # 8

F32 = mybir.dt.float32
BF16 = mybir.dt.bfloat16
F8 = mybir.dt.float8e4
DR = mybir.MatmulPerfMode.DoubleRow
EXP = mybir.ActivationFunctionType.Exp
IDENT = mybir.ActivationFunctionType.Identity
MULT = mybir.AluOpType.mult
INV32 = 1.0 / 32.0
BFNP = ml_dtypes.bfloat16
F8NP = ml_dtypes.float8_e4m3

LAST_RESULTS = None  # BassKernelResults of the most recent run (for test.py)


def _build_mha(tc, out_ap, in_aps):
    nc = tc.nc
    xth_d = in_aps["xth"]      # [128, 8, 2048] fp8 hi
    xtl_d = in_aps["xtl"]      # [128, 8, 2048] fp8 lo (residual)
    wqkh_d = in_aps["wqkh"]    # [128, 4, 2, 512] fp8 hi
    wqkl_d = in_aps["wqkl"]    # [128, 4, 2, 512] fp8 lo
    bq_d = in_aps["bq"]        # [128, 2] f32
    wvh_d = in_aps["wvh"]      # [128, 4, 2, 256] fp8 hi
    wvl_d = in_aps["wvl"]      # [128, 4, 2, 256] fp8 lo
    wout_d = in_aps["wout"]    # [128, 2, 1024] bf16
    mask_d = in_aps["mask"]    # [128, 2, 896] bf16

    with ExitStack() as ctx:
        ctx.enter_context(nc.allow_low_precision(reason="bf16/fp8 pipeline"))
        const = ctx.enter_context(tc.tile_pool(name="const", bufs=1))
        big = ctx.enter_context(tc.tile_pool(name="big", bufs=1))

        xth = big.tile([128, NDT, T], F8)
        xtl = big.tile([128, NDT, T], F8)
        # [32*(h%2)+j, h//2, A/B half, q/k, t]: per-head slices start at
        # partition 0 or 32 (base partition 96 is not addressable by the PE)
        qkt = big.tile([64, 2, 2, 2, T], F8)
        vp = big.tile([128, HL, NKT, 65], BF16)
        ot = big.tile([128, 2, T], BF16)

        wqkh = const.tile([128, 4, 2, 512], F8)
        wqkl = const.tile([128, 4, 2, 512], F8)
        wvh = const.tile([128, 4, 2, 256], F8)
        wvl = const.tile([128, 4, 2, 256], F8)
        wout = const.tile([128, 2, 1024], BF16)
        bq = const.tile([128, 2], F32)
        mask = const.tile([128, 2, 896], BF16)

        # HWDGE serializes ~625ns per DMA instruction: use few, large
        # transfers. Block-0 QKV inputs first, bulk x behind, weights last.
        nc.sync.dma_start(wqkh[:], wqkh_d)
        nc.sync.dma_start(xth[:, :, 0:TQ], xth_d[:, :, 0:TQ])
        nc.sync.dma_start(wqkl[:], wqkl_d)
        nc.sync.dma_start(xtl[:, :, 0:TQ], xtl_d[:, :, 0:TQ])
        nc.sync.dma_start(bq[:], bq_d)
        nc.sync.dma_start(mask[:], mask_d)
        nc.sync.dma_start(wvh[:], wvh_d)
        nc.sync.dma_start(wvl[:], wvl_d)
        nc.sync.dma_start(xth[:, :, TQ:], xth_d[:, :, TQ:])
        nc.sync.dma_start(xtl[:, :, TQ:], xtl_d[:, :, TQ:])
        nc.sync.dma_start(wout[:], wout_d)
        nc.gpsimd.memset(vp[:, :, :, 64], 1.0)

        with ExitStack() as pctx:
            ps = pctx.enter_context(tc.tile_pool(name="ps", bufs=2,
                                                 space="PSUM"))
            ps_av = pctx.enter_context(tc.tile_pool(name="ps_av", bufs=1,
                                                    space="PSUM"))
            ptp_p = pctx.enter_context(tc.tile_pool(name="ptp", bufs=8))
            nrm_p = pctx.enter_context(tc.tile_pool(name="nrm", bufs=4))
            ob_p = pctx.enter_context(tc.tile_pool(name="ob", bufs=6))

            def emit_fb(tt, fb):
                """One 128-col projection block (QA/QB/KA/KB) of token block
                tt via split-fp8 DoubleRow (hi*Whi + lo*Whi + hi*Wlo),
                evacuated into the fp8 qkt layout."""
                pq = ps.tile([128, TQ], F32, tag="pq", name=f"pq_{tt}_{fb}")
                cols = slice(fb * 128, (fb + 1) * 128)
                tb = slice(tt * TQ, (tt + 1) * TQ)
                terms = [(wqkh, xth), (wqkl, xth), (wqkh, xtl)]
                for ti, (w, xx) in enumerate(terms):
                    for j in range(4):
                        nc.tensor.matmul(
                            pq[:], w[:, j, :, cols], xx[:, 2 * j:2 * j + 2, tb],
                            start=(ti == 0 and j == 0),
                            stop=(ti == 2 and j == 3), perf_mode=DR)
                half, qk = fb & 1, fb >> 1
                # the two 64-row halves evacuate on different engines in
                # parallel so the PSUM bank frees in one evac latency
                for hp in range(2):
                    dst = qkt[:, hp, half, qk, tt * TQ:(tt + 1) * TQ]
                    src = pq[64 * hp:64 * hp + 64, :]
                    bqs = bq[64 * hp:64 * hp + 64, half:half + 1]
                    if qk == 0:
                        if hp == 0:
                            # out = in/32 + bq (Identity shares exp's table)
                            nc.scalar.activation(dst, src, IDENT, bias=bqs,
                                                 scale=INV32)
                        else:
                            nc.vector.tensor_scalar(
                                dst, src, INV32, bqs,
                                mybir.AluOpType.mult, mybir.AluOpType.add)
                    else:
                        if hp == 0:
                            nc.scalar.mul(dst, src, INV32)
                        else:
                            nc.vector.tensor_scalar_mul(dst, src, INV32)

            def emit_v(tt, ts):
                pv = ps.tile([128, 256], F32, tag="pq", name=f"pv_{tt}_{ts}")
                tb = slice((4 * tt + ts) * 128, (4 * tt + ts + 1) * 128)
                terms = [(xth, wvh), (xth, wvl), (xtl, wvh)]
                for ti, (xx, w) in enumerate(terms):
                    for j in range(4):
                        nc.tensor.matmul(
                            pv[:], xx[:, 2 * j:2 * j + 2, tb], w[:, j],
                            start=(ti == 0 and j == 0),
                            stop=(ti == 2 and j == 3), perf_mode=DR)
                nc.vector.tensor_scalar_mul(
                    vp[:, :, 4 * tt + ts, 0:64],
                    pv[:].rearrange("p (h e) -> p h e", e=HD), INV32)

            def emit_scores(a, qi, kt):
                """fp8 DoubleRow QK^T for head pair a at (qi, kt), one exp
                over both heads, diagonal mask multiply. Returns (ptp, c_lo).
                """
                rr = kt - 4 * qi
                c_lo = 0 if rr < 0 else 128 * rr
                s = ps.tile([128, 2, TQ], F32, tag="s", name=f"s_{a}_{qi}_{kt}")
                for i in range(2):
                    nc.tensor.matmul(
                        s[:, i, c_lo:],
                        qkt[32 * i:32 * i + 32, a, :, 1,
                            kt * TK:(kt + 1) * TK],
                        qkt[32 * i:32 * i + 32, a, :, 0,
                            qi * TQ + c_lo:(qi + 1) * TQ],
                        start=True, stop=True, perf_mode=DR)
                ptp = ptp_p.tile([128, 2, TQ], BF16, tag="pt",
                                 name=f"pt_{a}_{qi}_{kt}")
                nc.scalar.activation(ptp[:, :, c_lo:], s[:, :, c_lo:], EXP)
                if rr >= 0:
                    c0 = (3 - rr) * 128
                    nc.vector.tensor_tensor(
                        ptp[:, :, c_lo:c_lo + 128], ptp[:, :, c_lo:c_lo + 128],
                        mask[:, :, c0 + c_lo:c0 + c_lo + 128], MULT)
                return ptp, c_lo

            def emit_c(a, qi, fillers, rem_iters, pre=None, prefetch=None):
                """rem_iters: kt iterations left in this tt including this
                pair's — paces the filler drain across both pairs. `pre` is
                this pair's already-prefetched kt=0 scores; `prefetch` emits
                the NEXT pair's kt=0 scores during our last iteration so the
                pair boundary never exposes a full exp latency. Returns the
                prefetched scores for the next pair."""
                av = [ps_av.tile([65, TQ], F32, tag=f"av{i}",
                                 name=f"av{i}_{a}_{qi}") for i in range(2)]
                nkt = 4 * qi + 4
                pts, c_lo = pre if pre is not None else emit_scores(a, qi, 0)
                nxt_pre = None
                for kt in range(nkt):
                    if kt + 1 < nkt:
                        nxt = emit_scores(a, qi, kt + 1)
                    else:
                        nxt = (None, 0)
                        if prefetch is not None:
                            nxt_pre = prefetch()
                    n_pop = min(len(fillers),
                                -(-len(fillers) // max(1, rem_iters)))
                    rem_iters -= 1
                    for _ in range(n_pop):
                        fillers.popleft()()
                    for i in range(2):
                        h = 2 * a + i
                        nc.tensor.matmul(
                            av[i][0:65, c_lo:], vp[:, h, kt, :],
                            pts[:, i, c_lo:],
                            start=(kt == 0), stop=(kt == nkt - 1),
                            skip_group_check=True)
                    pts, c_lo = nxt
                # Evacuate av to SBUF immediately so the PSUM banks free for
                # the next pair; normalize off the critical path from SBUF.
                # Rows 0:64 are O^T, row 64 the softmax denominator. The last
                # block's banks are never reused — normalize from PSUM
                # directly to shorten the tail chain.
                last = qi == NQT - 1
                if last:
                    avs = av
                else:
                    avs = []
                    for i in range(2):
                        t = nrm_p.tile([65, TQ], F32, tag=f"avs{i}",
                                       name=f"avs{i}_{a}_{qi}")
                        nc.vector.tensor_copy(t[:], av[i][0:65, :])
                        avs.append(t)
                bcs = []
                for i in range(2):
                    rec = nrm_p.tile([1, TQ], F32, tag="rec",
                                     name=f"rec_{a}_{qi}_{i}")
                    nc.vector.reciprocal(rec[:], avs[i][64:65, :])
                    bc = nrm_p.tile([64, TQ], F32, tag="bc",
                                    name=f"bc_{a}_{qi}_{i}")
                    nc.gpsimd.partition_broadcast(bc[:], rec[:])
                    bcs.append(bc)
                if last:
                    # chunk per 128-token slice on DVE so the final
                    # out-projection units can start per-slice
                    for cs in range(4):
                        for i in range(2):
                            sl = slice(cs * 128, (cs + 1) * 128)
                            nc.vector.tensor_tensor(
                                ot[64 * i:64 * i + 64, a,
                                   qi * TQ + cs * 128:qi * TQ + (cs + 1) * 128],
                                avs[i][0:64, sl], bcs[i][:, sl], MULT)
                else:
                    for i in range(2):
                        nc.vector.tensor_tensor(
                            ot[64 * i:64 * i + 64, a,
                               qi * TQ:(qi + 1) * TQ],
                            avs[i][0:64, :], bcs[i][:], MULT)
                return nxt_pre

            def po_fillers(qi):
                def emit_po(ts, dt, ob):
                    po = ps.tile([128, TQ], F32, tag="pq",
                                 name=f"po_{ts}_{dt}")
                    for ft in range(2):
                        nc.tensor.matmul(
                            po[:], ot[:, ft, ts * 128:(ts + 1) * 128],
                            wout[:, ft, dt * 512:(dt + 1) * 512],
                            start=(ft == 0), stop=(ft == 1))
                    nc.vector.tensor_copy(ob[:, dt * 512:(dt + 1) * 512],
                                          po[:])
                    if dt == 1:
                        # one DMA per 128-token row (HWDGE overhead halves)
                        nc.sync.dma_start(out_ap[ts * 128:(ts + 1) * 128, :],
                                          ob[:])
                obs = {}
                def unit(ts, dt):
                    if dt == 0:
                        obs[ts] = ob_p.tile([128, 2 * TQ], BF16, tag="ob",
                                            name=f"ob_{ts}")
                    emit_po(ts, dt, obs[ts])
                return [
                    (lambda ts=ts, dt=dt: unit(ts, dt))
                    for ts in range(4 * qi, 4 * qi + 4) for dt in range(2)
                ]

            def qkv_fillers(tt):
                return ([(lambda fb=fb: emit_fb(tt, fb)) for fb in range(4)]
                        + [(lambda ts=ts: emit_v(tt, ts)) for ts in range(4)])

            # Block 0's Q/K projections run up front; everything else (V(0),
            # later blocks' QKV, all deferrable out-projections) slots into
            # emit_c's filler gaps so the PE never idles on the exp -> AV
            # dependency. The out-projections all land in the last block,
            # whose long kt loops otherwise run out of filler work.
            for fb in range(4):
                emit_fb(0, fb)
            pre = None
            for tt in range(NQT):
                fl = deque()
                if tt == 0:
                    fl.extend([(lambda ts=ts: emit_v(0, ts))
                               for ts in range(4)])
                if tt + 1 < NQT:
                    fl.extend(qkv_fillers(tt + 1))
                else:
                    for qi in range(NQT - 1):
                        fl.extend(po_fillers(qi))
                pre = emit_c(0, tt, fl, 2 * (4 * tt + 4), pre,
                             prefetch=lambda tt=tt: emit_scores(1, tt, 0))
                nxt_pf = ((lambda tt=tt: emit_scores(0, tt + 1, 0))
                          if tt + 1 < NQT else None)
                pre = emit_c(1, tt, fl, 4 * tt + 4, pre, prefetch=nxt_pf)
                while fl:
                    fl.popleft()()
            for f in po_fillers(NQT - 1):
                f()


_CACHE = {}


def _program():
    if "nc" in _CACHE:
        return _CACHE["nc"]
    nc = bacc.Bacc("TRN2", target_bir_lowering=False, debug=False)
    ins = {
        "xth": nc.dram_tensor("xth", [128, NDT, T], F8,
                              kind="ExternalInput").ap(),
        "xtl": nc.dram_tensor("xtl", [128, NDT, T], F8,
                              kind="ExternalInput").ap(),
        "wqkh": nc.dram_tensor("wqkh", [128, 4, 2, 512], F8,
                               kind="ExternalInput").ap(),
        "wqkl": nc.dram_tensor("wqkl", [128, 4, 2, 512], F8,
                               kind="ExternalInput").ap(),
        "bq": nc.dram_tensor("bq", [128, 2], F32, kind="ExternalInput").ap(),
        "wvh": nc.dram_tensor("wvh", [128, 4, 2, 256], F8,
                              kind="ExternalInput").ap(),
        "wvl": nc.dram_tensor("wvl", [128, 4, 2, 256], F8,
                              kind="ExternalInput").ap(),
        "wout": nc.dram_tensor("wout", [128, 2, 1024], BF16,
                               kind="ExternalInput").ap(),
        "mask": nc.dram_tensor("mask", [128, 2, 896], BF16,
                               kind="ExternalInput").ap(),
    }
    out = nc.dram_tensor("out", [T, D], BF16, kind="ExternalOutput").ap()
    with tile.TileContext(nc) as tc:
        _build_mha(tc, out, ins)
    nc.compile()
    _CACHE["nc"] = nc
    return nc


def _in_maps(x, Wqkv, bqkv, Wout):
    x = np.asarray(x, dtype=np.float32)
    Wqkv = np.asarray(Wqkv, dtype=np.float32)
    bqkv = np.asarray(bqkv, dtype=np.float32)
    Wout = np.asarray(Wout, dtype=np.float32)
    scale = np.float32(1.0 / np.sqrt(HD))
    maskbase = (np.arange(128)[:, None] <= np.arange(896)[None, :] - 384)
    mask = np.ascontiguousarray(
        np.broadcast_to(maskbase[:, None, :], (128, 2, 896))).astype(BFNP)
    maps = []
    for c in range(NCORES):
        b, hg = c // 4, c % 4
        hs = [4 * hg + i for i in range(HL)]
        # [1024, 256] per-projection slices for this head group
        q_cols = np.concatenate(
            [Wqkv[:, h * HD:(h + 1) * HD] for h in hs], axis=1) * scale
        k_cols = np.concatenate(
            [Wqkv[:, D + h * HD:D + (h + 1) * HD] for h in hs], axis=1)
        v_cols = np.concatenate(
            [Wqkv[:, 2 * D + h * HD:2 * D + (h + 1) * HD] for h in hs], axis=1)
        # A/B half split: [1024, 4 heads, 2 halves, 32] -> QA|QB / KA|KB
        qr = q_cols.reshape(D, HL, 2, 32)
        kr = k_cols.reshape(D, HL, 2, 32)
        wqk_full = np.concatenate(
            [qr[:, :, 0].reshape(D, 128), qr[:, :, 1].reshape(D, 128),
             kr[:, :, 0].reshape(D, 128), kr[:, :, 1].reshape(D, 128)],
            axis=1)                                    # [1024, 512]

        def w_split(w):
            """x32 scale (keeps the lo residual in fp8 normal range), then
            fp8 hi/lo split in the [128, 4, 2, C] DoubleRow layout."""
            w = w * np.float32(32.0)
            C = w.shape[1]
            hi = w.astype(F8NP)
            lo = (w - hi.astype(np.float32)).astype(F8NP)
            def lay(a):
                return np.ascontiguousarray(
                    a.reshape(4, 2, 128, C).transpose(2, 0, 1, 3))
            return lay(hi), lay(lo)

        wqkh, wqkl = w_split(wqk_full)
        wvh, wvl = w_split(v_cols)
        bq_cols = np.concatenate(
            [bqkv[h * HD:(h + 1) * HD] for h in hs]) * scale
        bqr = bq_cols.reshape(HL, 2, 32)
        bq = np.ascontiguousarray(
            np.stack([bqr[:, 0].reshape(128), bqr[:, 1].reshape(128)],
                     axis=1)).astype(np.float32)       # [128, 2]
        wo = np.concatenate([Wout[h * HD:(h + 1) * HD, :] for h in hs], axis=0)
        wo = np.ascontiguousarray(
            wo.reshape(2, 128, D).transpose(1, 0, 2)).astype(BFNP)
        xtf = np.ascontiguousarray(
            x[b].T.reshape(NDT, 128, T).transpose(1, 0, 2))
        xth = xtf.astype(F8NP)
        xtl = (xtf - xth.astype(np.float32)).astype(F8NP)
        maps.append({
            "xth": xth,
            "xtl": xtl,
            "wqkh": wqkh,
            "wqkl": wqkl,
            "bq": bq,
            "wvh": wvh,
            "wvl": wvl,
            "wout": wo,
            "mask": mask,
        })
    return maps


def kernel(x, Wqkv, bqkv, Wout, bout):
    global LAST_RESULTS
    nc = _program()
    maps = _in_maps(x, Wqkv, bqkv, Wout)
    res = run_bass_kernel_spmd(nc, maps, list(range(NCORES)))
    LAST_RESULTS = res
    bqkv = np.asarray(bqkv, dtype=np.float32)
    bout = np.asarray(bout, dtype=np.float32)
    # V bias folded: softmax weights sum to 1, so out += bv @ Wout exactly.
    bout_folded = bout + np.asarray(Wout, np.float32).T @ bqkv[2 * D:]
    out = np.empty((B, T, D), dtype=np.float32)
    for b in range(B):
        acc = np.asarray(res.results[4 * b]["out"], np.float32)
        for hg in range(1, 4):
            acc = acc + np.asarray(res.results[4 * b + hg]["out"], np.float32)
        out[b] = acc + bout_folded[None, :]
    return out
